# revision 1
# baseline (speedup 1.0000x reference)
"""Windowed multi-head attention (B=128 windows, N=512, C=256, H=8) on 8 TRN2 NeuronCores.

Strategy: data-parallel over windows (16 per core). Per window:
  qkv^T = W_qkv @ x^T (bf16 matmuls, feature-major out; q/k packed 3 heads
  per 128-partition block since SBUF base partitions are limited to
  0/32/64), v also computed token-major; per head: scores S^T[j, q]
  (bf16), exp on ScalarE (PSUM -> SBUF bf16), AV with P as stationary and
  [v | 1]-augmented moving operand (softmax denominator comes out as
  column 32), normalize on VectorE, PE-transpose attention output,
  proj (bf16) -> out^T. Evacuations are split between VectorE and
  ScalarE (Identity-with-bias shares the exp table set).
The program is software-pipelined: window w+1's qkv phase is emitted
before window w's attention heads so the scheduler overlaps them.
All layout transforms (transposes / shard split+gather) happen in numpy.
"""
import os
import sys

sys.path.insert(0, "/opt/trn_rl_repo")

import numpy as np
import ml_dtypes
from contextlib import ExitStack

N_CORES = 8
B, N, C = 128, 512, 256
H, HD = 8, 32
W = B // N_CORES  # windows per core


def make_stages(nc, pools, consts, xt_d, ot_d):
    import concourse.bass as bass
    from concourse import mybir

    F32 = mybir.dt.float32
    F32R = mybir.dt.float32r
    BF16 = mybir.dt.bfloat16
    MMDT = BF16 if os.environ.get("KERNEL_MM_DTYPE", "bf16") == "bf16" else F32R
    Exp = mybir.ActivationFunctionType.Exp
    Ident = mybir.ActivationFunctionType.Identity

    (xpool, qkpool, vpool, ppool, stpool, mmout, avpool, recpool,
     apool, atpool, finpool) = pools
    wq_sb, bq_sb, bv_sb, wp_sb, bp_sb, ident = consts

    def stage_qkv(iv):
        """Load x^T, produce qk (feature-major, bf16) and vaug (token-major)."""
        xw = xpool.tile([128, 2, 512], MMDT, tag="xw")
        nc.sync.dma_start(out=xw, in_=xt_d[iv])

        qk = []
        for mb in range(6):
            qkt = qkpool.tile([128, 512], BF16, tag=f"qk{mb}", name=f"qk{mb}")
            qk.append(qkt)
            ps = mmout.tile([128, 512], F32, tag="mm", name="qkps")
            for _rep in range(int(os.environ.get("KERNEL_QKV_REPS", "1"))):
                for cb in range(2):
                    nc.tensor.matmul(
                        ps, wq_sb[:, cb, 128 * mb:128 * mb + 128], xw[:, cb, :],
                        start=(cb == 0), stop=(cb == 1))
            if os.environ.get("KERNEL_EVAC", "mix") == "mix" and mb % 2 == 1:
                nc.scalar.activation(out=qkt[:], in_=ps, func=Ident,
                                     bias=bq_sb[:, mb:mb + 1], scale=1.0)
            else:
                nc.vector.tensor_scalar_add(qkt[:], ps, bq_sb[:, mb:mb + 1])

        vaug = vpool.tile([128, 4, 8, 33], BF16, tag="vaug")
        nc.gpsimd.memset(vaug[:, :, :, 32:33], 1.0)
        if os.environ.get("KERNEL_VPAIR", "1") == "1":
            for tp in range(2):
                ps = mmout.tile([128, 512], F32, tag="mm", name="vps")
                for half in range(2):
                    for cb in range(2):
                        nc.tensor.matmul(
                            ps[:, 256 * half:256 * half + 256],
                            xw[:, cb, 128 * (2 * tp + half):128 * (2 * tp + half) + 128],
                            wq_sb[:, cb, 768:1024],
                            start=(cb == 0), stop=(cb == 1))
                bvb = bass.AP(tensor=bv_sb.tensor, offset=bv_sb.offset,
                              ap=[[bv_sb.ap[0][0], 128], [0, 2],
                                  [32, 8], [1, 32]])
                nc.vector.tensor_add(
                    vaug[:, 2 * tp:2 * tp + 2, :, 0:32],
                    ps.rearrange("p (t h d) -> p t h d", t=2, h=8),
                    bvb)
        else:
            for tb in range(4):
                ps = mmout.tile([128, 256], F32, tag="mm")
                for cb in range(2):
                    nc.tensor.matmul(
                        ps, xw[:, cb, 128 * tb:128 * tb + 128], wq_sb[:, cb, 768:1024],
                        start=(cb == 0), stop=(cb == 1))
                nc.vector.tensor_add(
                    vaug[:, tb, :, 0:32],
                    ps.rearrange("p (h d) -> p h d", h=8),
                    bv_sb.rearrange("p (h d) -> p h d", h=8))
        return qk, vaug

    def new_atto():
        atto_lo = apool.tile([128, 4, 128], BF16, tag="attolo", name="attolo")
        atto_hi = apool.tile([128, 4, 128], BF16, tag="attohi", name="attohi")
        return [atto_lo, atto_hi]

    def stage_head(h, qk, vaug, atto):
        if True:
            a, g = 32 * (h % 3), h // 3
            st0 = stpool.tile([128, 2, 512], F32, tag="st")
            st1 = stpool.tile([128, 2, 512], F32, tag="st")
            st = [st0, st1]
            nsc = 1 if os.environ.get("KERNEL_SC_CUT", "0") == "1" else 4
            sc_reps = int(os.environ.get("KERNEL_SC_REPS", "1"))
            for jb in range(nsc):
                for _ in range(sc_reps):
                    nc.tensor.matmul(
                        st[jb // 2][:, jb % 2, :],
                        qk[3 + g][a:a + 32, 128 * jb:128 * jb + 128],
                        qk[g][a:a + 32, :],
                        start=True, stop=True)
            if nsc == 1:
                st[1] = st[0]
            ph = ppool.tile([128, 4, 512], BF16, tag="ph")
            if os.environ.get("KERNEL_NO_EXP", "0") == "1":
                nc.gpsimd.memset(ph[:, 0:2, :], 0.002)
                nc.gpsimd.memset(ph[:, 2:4, :], 0.002)
            else:
                nc.scalar.activation(out=ph[:, 0:2, :], in_=st[0], func=Exp)
                nc.scalar.activation(out=ph[:, 2:4, :], in_=st[1], func=Exp)
            av = avpool.tile([128, 4, 33], F32, tag="avtx")
            njb = 1 if os.environ.get("KERNEL_AV_CUT", "0") == "1" else 4
            for qb in range(4):
                for jb in range(njb):
                    nc.tensor.matmul(
                        av[:, qb, :],
                        ph[:, jb, 128 * qb:128 * qb + 128],
                        vaug[:, jb, h, :],
                        start=(jb == 0), stop=(jb == njb - 1))
            rh = recpool.tile([128, 4], F32, tag="rec")
            nc.vector.reciprocal(rh, av[:, :, 32])
            rb = bass.AP(tensor=rh.tensor, offset=rh.offset,
                         ap=[[rh.ap[0][0], 128], [rh.ap[1][0], 4], [0, 32]])
            nc.vector.tensor_mul(
                atto[h // 4][:, :, 32 * (h % 4):32 * (h % 4) + 32],
                av[:, :, 0:32], rb)

    def stage_tail(iv, atto):
        at = atpool.tile([128, 2, 512], BF16, tag="at")
        for cb in range(2):
            tx = avpool.tile([128, 4, 128], BF16, tag="avtx")
            for tb in range(4):
                nc.tensor.transpose(
                    tx[:, tb, :], atto[cb][:, tb, :], ident)
            if os.environ.get("KERNEL_EVAC", "mix") == "mix":
                nc.scalar.copy(at[:, cb, :], tx)
            else:
                nc.vector.tensor_copy(at[:, cb, :], tx)
        for mb in range(2):
            ps = mmout.tile([128, 512], F32, tag="mm")
            for cb in range(2):
                nc.tensor.matmul(
                    ps, wp_sb[:, cb, 128 * mb:128 * mb + 128], at[:, cb, :],
                    start=(cb == 0), stop=(cb == 1))
            fin = finpool.tile([128, 512], F32, tag="fin")
            if os.environ.get("KERNEL_EVAC", "mix") == "mix":
                nc.scalar.activation(out=fin, in_=ps, func=Ident,
                                     bias=bp_sb[:, mb:mb + 1], scale=1.0)
            else:
                nc.vector.tensor_scalar_add(fin, ps, bp_sb[:, mb:mb + 1])
            nc.sync.dma_start(out=ot_d[iv, :, mb, :], in_=fin)

    return stage_qkv, new_atto, stage_head, stage_tail


def build_nc(n_windows=W, repeat=None):
    import concourse.bass as bass
    import concourse.tile as tile
    from concourse import bacc, mybir
    from concourse.masks import make_identity

    if repeat is None:
        repeat = int(os.environ.get("KERNEL_REPEAT", "1"))

    F32 = mybir.dt.float32
    F32R = mybir.dt.float32r
    BF16 = mybir.dt.bfloat16
    MMDT = BF16 if os.environ.get("KERNEL_MM_DTYPE", "bf16") == "bf16" else F32R

    nc = bacc.Bacc("TRN2", target_bir_lowering=False, debug=False,
                   num_devices=N_CORES)
    xt_d = nc.dram_tensor("xt", [n_windows, 128, 2, 512], MMDT,
                          kind="ExternalInput").ap()
    wq_d = nc.dram_tensor("wq", [128, 2, 1024], MMDT, kind="ExternalInput").ap()
    bq_d = nc.dram_tensor("bq", [128, 6], F32, kind="ExternalInput").ap()
    bv_d = nc.dram_tensor("bv", [128, 256], F32, kind="ExternalInput").ap()
    wp_d = nc.dram_tensor("wp", [128, 2, 256], BF16, kind="ExternalInput").ap()
    bp_d = nc.dram_tensor("bp", [128, 2], F32, kind="ExternalInput").ap()
    ot_d = nc.dram_tensor("ot", [n_windows, 128, 2, 512], F32,
                          kind="ExternalOutput").ap()

    with tile.TileContext(nc) as tc, ExitStack() as ctx:
        persist = ctx.enter_context(tc.tile_pool(name="persist", bufs=1))
        xpool = ctx.enter_context(tc.tile_pool(name="xpool", bufs=3))
        qkpool = ctx.enter_context(tc.tile_pool(name="qkpool", bufs=3))
        vpool = ctx.enter_context(tc.tile_pool(name="vpool", bufs=3))
        ppool = ctx.enter_context(tc.tile_pool(name="ppool", bufs=3))
        psum_b3 = os.environ.get("KERNEL_PSUM_B3", "0") == "1"
        stpool = ctx.enter_context(tc.tile_pool(name="stpool", bufs=2, space="PSUM"))
        mmout = ctx.enter_context(tc.tile_pool(
            name="mmout", bufs=3 if psum_b3 else 2, space="PSUM"))
        avpool = ctx.enter_context(tc.tile_pool(
            name="avpool", bufs=1 if psum_b3 else 2, space="PSUM"))
        recpool = ctx.enter_context(tc.tile_pool(name="recpool", bufs=8))
        apool = ctx.enter_context(tc.tile_pool(name="apool", bufs=3))
        atpool = ctx.enter_context(tc.tile_pool(name="atpool", bufs=3))
        finpool = ctx.enter_context(tc.tile_pool(name="finpool", bufs=4))

        wq_sb = persist.tile([128, 2, 1024], MMDT, tag="wq")
        nc.sync.dma_start(out=wq_sb, in_=wq_d)
        bq_sb = persist.tile([128, 6], F32, tag="bq")
        nc.sync.dma_start(out=bq_sb, in_=bq_d)
        bv_sb = persist.tile([128, 256], F32, tag="bv")
        nc.sync.dma_start(out=bv_sb, in_=bv_d)
        wp_sb = persist.tile([128, 2, 256], BF16, tag="wp")
        nc.sync.dma_start(out=wp_sb, in_=wp_d)
        bp_sb = persist.tile([128, 2], F32, tag="bp")
        nc.sync.dma_start(out=bp_sb, in_=bp_d)
        ident = persist.tile([128, 128], BF16, tag="id")
        make_identity(nc, ident)

        pools = (xpool, qkpool, vpool, ppool, stpool, mmout, avpool, recpool,
                 apool, atpool, finpool)
        consts = (wq_sb, bq_sb, bv_sb, wp_sb, bp_sb, ident)
        stage_qkv, new_atto, stage_head, stage_tail = make_stages(
            nc, pools, consts, xt_d, ot_d)

        def full_pass():
            # software pipeline: emit `split` heads of window w first so the
            # scalar engine has exp work queued, then qkv(w+1), then the rest
            split = int(os.environ.get("KERNEL_PIPE_SPLIT", "0"))
            pending = stage_qkv(0)
            for w in range(n_windows):
                qk, vaug = pending
                atto = new_atto()
                for h in range(split):
                    stage_head(h, qk, vaug, atto)
                pending = stage_qkv(w + 1) if w + 1 < n_windows else None
                for h in range(split, H):
                    stage_head(h, qk, vaug, atto)
                stage_tail(w, atto)

        body_passes = int(os.environ.get("KERNEL_BODY_PASSES", "1"))
        if repeat > 1:
            def rep_body(r):
                for _ in range(body_passes):
                    full_pass()
            tc.For_i_unrolled(0, repeat, 1, rep_body, max_unroll=1)
        else:
            full_pass()

    nc.compile()
    return nc


def prep_inputs(x, qkv_w, qkv_b, proj_w, proj_b, n_windows_per_core=W,
                n_cores=N_CORES):
    """Shard + lay out inputs for the per-core DRAM parameters."""
    x = np.asarray(x, dtype=np.float32)
    qkv_w = np.asarray(qkv_w, dtype=np.float32)
    qkv_b = np.asarray(qkv_b, dtype=np.float32)
    proj_w = np.asarray(proj_w, dtype=np.float32)
    proj_b = np.asarray(proj_b, dtype=np.float32)

    sc = HD ** -0.5
    qkv_w_s = qkv_w.copy()
    qkv_w_s[:C] *= sc
    qkv_b_s = qkv_b.copy()
    qkv_b_s[:C] *= sc

    # q/k feature blocks: 3 heads (96 feats) per 128-col block, zero padded,
    # so every head starts at partition offset 0/32/64 (offset 96 is not
    # encodable in SBUF access patterns).
    qpad = np.zeros((384, C), np.float32)
    kpad = np.zeros((384, C), np.float32)
    bqpad = np.zeros(768, np.float32)
    for b in range(3):
        lo, hi = 96 * b, min(96 * b + 96, C)
        qpad[128 * b:128 * b + hi - lo] = qkv_w_s[lo:hi]
        kpad[128 * b:128 * b + hi - lo] = qkv_w_s[C + lo:C + hi]
        bqpad[128 * b:128 * b + hi - lo] = qkv_b_s[lo:hi]
        bqpad[384 + 128 * b:384 + 128 * b + hi - lo] = qkv_b_s[C + lo:C + hi]
    wq_all = np.concatenate([qpad, kpad, qkv_w_s[2 * C:]], axis=0)  # [1024, C]
    wq = np.ascontiguousarray(wq_all.reshape(1024, 2, 128).transpose(2, 1, 0))
    bq = np.ascontiguousarray(bqpad.reshape(6, 128).T)
    bv = np.ascontiguousarray(np.broadcast_to(qkv_b[2 * C:], (128, C)))
    wp = np.ascontiguousarray(
        proj_w.reshape(C, 2, 128).transpose(2, 1, 0)).astype(ml_dtypes.bfloat16)
    bp = np.ascontiguousarray(proj_b.reshape(2, 128).T)

    if os.environ.get("KERNEL_MM_DTYPE", "bf16") == "bf16":
        wq = wq.astype(ml_dtypes.bfloat16)
    in_maps = []
    for c in range(n_cores):
        xs = x[c * n_windows_per_core:(c + 1) * n_windows_per_core]
        xt = np.ascontiguousarray(
            xs.reshape(n_windows_per_core, N, 2, 128).transpose(0, 3, 2, 1))
        if os.environ.get("KERNEL_MM_DTYPE", "bf16") == "bf16":
            xt = xt.astype(ml_dtypes.bfloat16)
        in_maps.append(
            {"xt": xt, "wq": wq, "bq": bq, "bv": bv, "wp": wp, "bp": bp})
    return in_maps


def assemble_output(results, n_windows_per_core=W, n_cores=N_CORES):
    outs = []
    for c in range(n_cores):
        ot = results[c]["ot"]  # [W, 128, 2, 512]
        y = ot.transpose(0, 3, 2, 1).reshape(n_windows_per_core, N, C)
        outs.append(y)
    return np.ascontiguousarray(np.concatenate(outs, axis=0), dtype=np.float32)


_NC_CACHE = {}
LAST_EXEC_TIME_NS = None


def kernel(x, qkv_w, qkv_b, proj_w, proj_b):
    global LAST_EXEC_TIME_NS
    from concourse.bass_utils import run_bass_kernel_spmd

    if "nc" not in _NC_CACHE:
        _NC_CACHE["nc"] = build_nc(W, repeat=1)
    nc = _NC_CACHE["nc"]

    in_maps = prep_inputs(x, qkv_w, qkv_b, proj_w, proj_b)
    res = run_bass_kernel_spmd(nc, in_maps, core_ids=list(range(N_CORES)))
    LAST_EXEC_TIME_NS = res.exec_time_ns
    return assemble_output(res.results)

